# revision 23
# baseline (speedup 1.0000x reference)
"""GRUCell + LayerNorm readout fused Bass kernel for Trainium2 (8 NeuronCores).

Problem: B=8192, D=H=O=1024 fp32.
    r = sigmoid(x@Wir + bir + h@Whr)
    z = sigmoid(x@Wiz + biz + h@Whz)
    n = tanh(x@Win + bin_ + r*(h@Whn + bhn))
    new_h = (1-z)*n + z*h
    out = (LayerNorm(new_h)*ln_scale + ln_bias) @ Wout + bout

Strategy:
  - Data-parallel over batch: core c gets rows [c*1024, (c+1)*1024); weights
    replicated. No collectives.
  - Everything computed in the transposed domain: activations live as
    [feature, batch] so matmuls take the weights in natural [k, h] layout as
    the stationary operand and xT/hT as the moving operand, and the per-h gate
    biases become per-partition activation biases. Host passes xT/hT and
    transposes the outputs back.
  - All GEMMs in bf16 (1 col/cycle on the PE like f32r, but half the HBM
    traffic and FWL-fast weight loads). Weights are pre-tiled on the host so
    every weight DMA is a contiguous [128, 1024]-per-partition block.
  - Matmuls are emitted k-major, gate-major with both batch chunks adjacent so
    consecutive matmuls share a stationary (one LDWEIGHTS per two matmuls);
    the 8 gate accumulators occupy all 8 PSUM banks.
  - LayerNorm reduces over h (= partition dim): per-tile partials accumulate
    elementwise on DVE, one ones-column reduce matmul per stat at the end.
    The normalize-then-matmul is algebraically folded:
        LN(new_h) @ (ln_scale*Wout) + (ln_bias@Wout + bout)
      = rstd[b]*( new_h@WoutF - mu[b]*colsum[o] ) + boutF[o]
    with WoutF = ln_scale[:,None]*Wout (host), colsum = ln_scale@Wout (host),
    boutF = bout + ln_bias@Wout (host); the mu*colsum term is a K=1 rank-1
    matmul into the same PSUM accumulator.
  - new_h / out are stored to HBM in bf16 and upcast on the host (outputs are
    checked at 2e-2; bf16 rounding is ~4e-3 scale-relative).
"""

import sys
from contextlib import ExitStack

sys.path.insert(0, "/opt/trn_rl_repo")

import ml_dtypes
import numpy as np

import concourse.bacc as bacc
import concourse.mybir as mybir
import concourse.tile as tile
from concourse import bass_isa, bass_utils

B, D, H, O = 8192, 1024, 1024, 1024
NCORES = 8
BL = B // NCORES          # batch rows per core
P = 128                   # partitions
KT = D // P               # contraction tiles (8)
HT = H // P               # h output-partition tiles (8)
OT = O // P               # o output-partition tiles (8)
NB = 2                    # batch chunks per core (free dim 512)
NF = BL // NB             # free dim per chunk (512)
LN_EPS = 1e-6

F32 = mybir.dt.float32
F32R = mybir.dt.float32r
BF16 = mybir.dt.bfloat16
NP_BF16 = ml_dtypes.bfloat16

_COMPILED = None  # compiled Bacc module cache across calls
TRACE = False     # set by test harness to capture an NTFF profile
LAST_RES = None   # BassKernelResults of the last run (for the test harness)

XGATES = ("ir", "iz", "in")
HGATES = ("hr", "hz", "hn")


def _build():
    nc = bacc.Bacc("TRN2", target_bir_lowering=False, debug=False,
                   num_devices=NCORES)

    def din(name, shape, dt=BF16):
        return nc.dram_tensor(name, shape, dt, kind="ExternalInput").ap()

    def dout(name, shape, dt=BF16):
        return nc.dram_tensor(name, shape, dt, kind="ExternalOutput").ap()

    xT_d = din("xT", [D, BL])
    hT_d = din("hT", [H, BL])
    # weights pre-tiled on host: [HT*P, KT*P] with [ht*P+p, t*P+h] =
    # W[t*P+p, ht*P+h]; a per-ht load is contiguous per partition.
    w_d = {g: din(f"W{g}", [HT * P, KT * P]) for g in XGATES + HGATES}
    # [P, KT*O] with [p, t*O+o] = WoutF[t*P+p, o]
    woutF_d = din("woutF", [P, KT * O])
    # bias vectors pre-arranged host-side as [P, HT] (contiguous per partition)
    bir_d = din("bir", [P, HT], F32)
    biz_d = din("biz", [P, HT], F32)
    bin_d = din("bin", [P, HT], F32)
    bhn_d = din("bhn", [P, HT], F32)
    boutF_d = din("boutF", [P, OT], F32)
    colsum_d = din("colsum", [1, O], F32R)
    ones_col_d = din("ones_col", [P, 1], F32R)
    ones_row_d = din("ones_row", [1, P], F32R)

    nhT_d = dout("nhT", [H, BL])
    outT_d = dout("outT", [O, BL])

    with tile.TileContext(nc) as tc, ExitStack() as ctx:
        singles = ctx.enter_context(tc.tile_pool(name="singles", bufs=1))
        wpool = ctx.enter_context(tc.tile_pool(name="wpool", bufs=2))
        gates = ctx.enter_context(tc.tile_pool(name="gates", bufs=1))
        rows = ctx.enter_context(tc.tile_pool(name="rows", bufs=1))
        ps = ctx.enter_context(tc.tile_pool(name="ps", bufs=1, space="PSUM"))

        # ---- resident inputs, DMA-ordered to feed the PE ramp ---------------
        def kslice_tile(prefix, k):
            return singles.tile([P, BL], BF16, tag=f"{prefix}{k}",
                                name=f"{prefix}{k}")

        def load_w(g, ht):
            t = wpool.tile([P, KT, P], BF16, tag=f"w{g}", name=f"w{g}_{ht}")
            nc.sync.dma_start(t[:], w_d[g][ht * P:(ht + 1) * P, :].rearrange(
                "p (t h) -> p t h", t=KT))
            return t

        xT_sb, hT_sb = [], []
        for k in range(KT):
            xT_sb.append(kslice_tile("xk", k))
            hT_sb.append(kslice_tile("hk", k))

        # input slices ride the Scalar engine's DMA queue and weights the
        # Sync queue: two issue streams (~0.6us per DMA instruction) so the
        # prologue isn't serialized on a single queue.
        nc.scalar.dma_start(xT_sb[0][:], xT_d[0:P, :])
        w0 = {g: load_w(g, 0) for g in XGATES}
        for k in range(1, KT):
            nc.scalar.dma_start(xT_sb[k][:], xT_d[k * P:(k + 1) * P, :])
        w0["hr"] = load_w("hr", 0)
        w0["hz"] = load_w("hz", 0)
        w0["hn"] = load_w("hn", 0)
        for k in range(KT):
            nc.scalar.dma_start(hT_sb[k][:], hT_d[k * P:(k + 1) * P, :])

        def load_vec(ap_d, n, tag):
            t = singles.tile([P, n // P], F32, tag=tag, name=tag)
            nc.scalar.dma_start(t[:], ap_d)
            return t

        bir_sb = load_vec(bir_d, H, "bir_sb")
        biz_sb = load_vec(biz_d, H, "biz_sb")
        bin_sb = load_vec(bin_d, H, "bin_sb")
        bhn_sb = load_vec(bhn_d, H, "bhn_sb")
        boutF_sb = load_vec(boutF_d, O, "boutF_sb")
        colsum_sb = singles.tile([1, O], F32R)
        nc.scalar.dma_start(colsum_sb[:], colsum_d)
        ones_col = singles.tile([P, 1], F32R)
        nc.scalar.dma_start(ones_col[:], ones_col_d)
        ones_row = singles.tile([1, P], F32R)
        nc.scalar.dma_start(ones_row[:], ones_row_d)
        eps_sb = singles.tile([1, 1], F32)
        nc.vector.memset(eps_sb[:], LN_EPS)

        # ---- PE warmup: junk matmuls on a zeroed tile keep the HAM activity
        # monitor busy while the first weights/inputs stream in, so the real
        # matmul stream starts at the full 2.4GHz clock.
        warm_sb = singles.tile([P, NF], BF16, tag="warm", name="warm_sb")
        nc.vector.memset(warm_sb[:], 0.0)
        warm_ps = ps.tile([P, NF], F32, tag="gh1", name="warm_ps")
        for _ in range(16):
            nc.tensor.matmul(warm_ps[:], warm_sb[:, 0:P], warm_sb[:],
                             start=True, stop=True, skip_group_check=True)

        # per-(ht, bc) new_h tiles: tile-granular deps keep the readout
        # matmuls from waiting on the other batch-chunk's epilogue
        new_hT_sb = [[singles.tile([P, NF], BF16, tag=f"nh{ht}_{bc}",
                                   name=f"nh{ht}_{bc}") for bc in range(NB)]
                     for ht in range(HT)]
        s_acc = [singles.tile([P, NF], F32R, tag=f"s_acc{bc}",
                              name=f"s_acc{bc}") for bc in range(NB)]
        q_acc = [singles.tile([P, NF], F32R, tag=f"q_acc{bc}",
                              name=f"q_acc{bc}") for bc in range(NB)]

        # ---- phase 1: gates + new_h -----------------------------------------
        woutF_sb = singles.tile([P, KT, O], BF16)

        for ht in range(HT):
            hs = slice(ht * P, (ht + 1) * P)
            w_sb = w0 if ht == 0 else {g: load_w(g, ht)
                                       for g in XGATES + HGATES}
            if ht == 2:
                # readout weights: resident; loaded after the ramp-critical
                # input/gate-weight prefetches are in flight
                nc.sync.dma_start(woutF_sb[:], woutF_d.rearrange(
                    "p (t o) -> p t o", t=KT))

            pr = [ps.tile([P, NF], F32, tag=f"r{bc}", name=f"pr{bc}_{ht}")
                  for bc in range(NB)]
            pz = [ps.tile([P, NF], F32, tag=f"z{bc}", name=f"pz{bc}_{ht}")
                  for bc in range(NB)]
            pgi = [ps.tile([P, NF], F32, tag=f"gi{bc}", name=f"pgi{bc}_{ht}")
                   for bc in range(NB)]
            pgh = [ps.tile([P, NF], F32, tag=f"gh{bc}", name=f"pgh{bc}_{ht}")
                   for bc in range(NB)]

            bsl = [slice(bc * NF, (bc + 1) * NF) for bc in range(NB)]

            # bc-major: all of bc0's matmuls, then bc1's. bc0's PSUM groups
            # stop ~5us before the ht ends, so its activation/epilogue chain
            # (and at ht=7 the nh tile the readout needs) hides under bc1's
            # matmuls instead of stalling the next block.
            for bc in range(NB):
                for k in range(KT):
                    nc.tensor.matmul(pr[bc][:], w_sb["ir"][:, k, :],
                                     xT_sb[k][:, bsl[bc]],
                                     start=(k == 0), stop=False)
                    nc.tensor.matmul(pz[bc][:], w_sb["iz"][:, k, :],
                                     xT_sb[k][:, bsl[bc]],
                                     start=(k == 0), stop=False)
                    nc.tensor.matmul(pgi[bc][:], w_sb["in"][:, k, :],
                                     xT_sb[k][:, bsl[bc]],
                                     start=(k == 0), stop=(k == KT - 1))
            for bc in range(NB):
                for k in range(KT):
                    nc.tensor.matmul(pr[bc][:], w_sb["hr"][:, k, :],
                                     hT_sb[k][:, bsl[bc]],
                                     start=False, stop=(k == KT - 1))
                    nc.tensor.matmul(pz[bc][:], w_sb["hz"][:, k, :],
                                     hT_sb[k][:, bsl[bc]],
                                     start=False, stop=(k == KT - 1))
                    nc.tensor.matmul(pgh[bc][:], w_sb["hn"][:, k, :],
                                     hT_sb[k][:, bsl[bc]],
                                     start=(k == 0), stop=(k == KT - 1))

            for bc in range(NB):
                bs = bsl[bc]
                r_sb = gates.tile([P, NF], F32, tag="r_act")
                nc.scalar.activation(r_sb[:], pr[bc][:],
                                     mybir.ActivationFunctionType.Sigmoid,
                                     bias=bir_sb[:, ht:ht + 1])
                z_sb = gates.tile([P, NF], F32, tag="z_act")
                nc.scalar.activation(z_sb[:], pz[bc][:],
                                     mybir.ActivationFunctionType.Sigmoid,
                                     bias=biz_sb[:, ht:ht + 1])

                t_sb = gates.tile([P, NF], F32, tag="t")
                nc.vector.tensor_scalar(t_sb[:], pgh[bc][:],
                                        bhn_sb[:, ht:ht + 1],
                                        None, mybir.AluOpType.add)
                nc.vector.tensor_mul(t_sb[:], t_sb[:], r_sb[:])
                nc.vector.tensor_add(t_sb[:], t_sb[:], pgi[bc][:])
                n_sb = gates.tile([P, NF], F32, tag="r_act", name="n_sb")
                nc.scalar.activation(n_sb[:], t_sb[:],
                                     mybir.ActivationFunctionType.Tanh,
                                     bias=bin_sb[:, ht:ht + 1])

                u_sb = gates.tile([P, NF], F32, tag="u")
                nc.vector.tensor_tensor(u_sb[:], hT_sb[ht][:, bs],
                                        n_sb[:], mybir.AluOpType.subtract)
                nc.vector.tensor_mul(u_sb[:], z_sb[:], u_sb[:])
                nh = new_hT_sb[ht][bc][:]
                nc.vector.tensor_add(nh, n_sb[:], u_sb[:])

                # LN stat partials: elementwise accumulate over h-tiles (DVE),
                # cross-partition reduce later via a ones-column matmul.
                sq_sb = gates.tile([P, NF], F32R, tag="t", name="sq_sb")
                if ht == 0:
                    nc.vector.tensor_copy(s_acc[bc][:], nh)
                    nc.scalar.activation(q_acc[bc][:], nh,
                                         mybir.ActivationFunctionType.Square)
                else:
                    nc.vector.tensor_tensor(s_acc[bc][:],
                                            s_acc[bc][:].bitcast(F32), nh,
                                            mybir.AluOpType.add)
                    nc.scalar.activation(sq_sb[:], nh,
                                         mybir.ActivationFunctionType.Square)
                    nc.vector.tensor_tensor(q_acc[bc][:],
                                            q_acc[bc][:].bitcast(F32),
                                            sq_sb[:].bitcast(F32),
                                            mybir.AluOpType.add)

                # stores go through GpSimd's DMA queue so they never
                # head-of-line-block weight loads on the Sync queue
                nc.gpsimd.dma_start(nhT_d[hs, bs], nh)

        # ---- phase 2: LN scale factors + readout ----------------------------
        # bc=0 groups run first so the bc=1 stats chain hides under them; the
        # stats reduce-matmuls, broadcast matmuls, rank-1s, and epilogues are
        # all software-pipelined into the main matmul stream so the in-order
        # PE never stalls on the stats chain or cools down (HAM).
        red_tags = ("gi0", "gi1", "gi0", "gi1")
        nmu_row = {}
        rstd_row = {}
        rstd_bc = {}

        def emit_stats(bc):
            psum_s = ps.tile([1, NF], F32, tag=red_tags[2 * bc],
                             name=f"psum_s{bc}")
            nc.tensor.matmul(psum_s[:], ones_col[:], s_acc[bc][:],
                             start=True, stop=True)
            psum_q = ps.tile([1, NF], F32, tag=red_tags[2 * bc + 1],
                             name=f"psum_q{bc}")
            nc.tensor.matmul(psum_q[:], ones_col[:], q_acc[bc][:],
                             start=True, stop=True)

            nmu = rows.tile([1, NF], F32R, tag=f"nmu{bc}", name=f"nmu{bc}")
            nc.vector.tensor_scalar_mul(nmu[:], psum_s[:], -1.0 / H)
            nmu_row[bc] = nmu

            mu2 = gates.tile([1, NF], F32, tag="t", name=f"mu2_{bc}")
            nc.vector.tensor_mul(mu2[:], nmu[:].bitcast(F32), nmu[:].bitcast(F32))
            var = gates.tile([1, NF], F32, tag="u", name=f"var_{bc}")
            nc.vector.tensor_scalar_mul(var[:], psum_q[:], 1.0 / H)
            nc.vector.tensor_tensor(var[:], var[:], mu2[:],
                                    mybir.AluOpType.subtract)
            nc.scalar.activation(var[:], var[:],
                                 mybir.ActivationFunctionType.Sqrt,
                                 bias=eps_sb[:])
            rtmp = gates.tile([1, NF], F32, tag="t", name=f"rstdf{bc}")
            nc.vector.reciprocal_approx_fast(rtmp[:], var[:])
            rrow = gates.tile([1, NF], F32R, tag=("z_act", "r_act")[bc],
                              name=f"rstd{bc}")
            nc.vector.tensor_copy(rrow[:], rtmp[:])
            rstd_row[bc] = rrow

        po_tags = ("r0", "z0", "r1", "z1", "gh0", "gh1")
        PIPE = 5
        groups = [(ot, bc) for bc in range(NB) for ot in range(OT)]
        pending = {}

        def finalize(i):
            ot, bc = groups[i]
            po = pending.pop(i)
            os_ = slice(ot * P, (ot + 1) * P)
            bs = slice(bc * NF, (bc + 1) * NF)
            # -= mu[b] * colsum[o]  (rank-1, K=1)
            nc.tensor.matmul(po[:], colsum_sb[0:1, os_], nmu_row[bc][:],
                             start=False, stop=True)
            o_sb = gates.tile([P, NF], BF16, tag=("ob0", "ob1", "ob2")[i % 3],
                              name=f"o_{ot}_{bc}")
            nc.vector.tensor_mul(o_sb[:], po[:], rstd_bc[bc][:])
            nc.vector.tensor_scalar(o_sb[:], o_sb[:],
                                    boutF_sb[:, ot:ot + 1], None,
                                    mybir.AluOpType.add)
            # sync queue is idle during phase 2; keeps the gpsimd drain short
            nc.sync.dma_start(outT_d[os_, bs], o_sb[:])

        def emit_pb(bc):
            pb = ps.tile([P, NF], F32, tag=red_tags[bc], name=f"pb{bc}")
            nc.tensor.matmul(pb[:], ones_row[:], rstd_row[bc][:],
                             start=True, stop=True)
            rb = rows.tile([P, NF], F32, tag=f"rstd_bc{bc}",
                           name=f"rstd_bc{bc}")
            nc.vector.tensor_copy(rb[:], pb[:])
            rstd_bc[bc] = rb

        done = 0
        for i, (ot, bc) in enumerate(groups):
            po = ps.tile([P, NF], F32, tag=po_tags[i % len(po_tags)],
                         name=f"po_{ot}_{bc}")
            for k in range(HT):
                nc.tensor.matmul(po[:], woutF_sb[:, k, ot * P:(ot + 1) * P],
                                 new_hT_sb[k][bc][:],
                                 start=(k == 0), stop=False)
            pending[i] = po
            if i == 1:
                emit_stats(0)
            elif i == 3:
                emit_stats(1)
            elif i == 5:
                emit_pb(0)
            elif i == 7:
                emit_pb(1)
            if i >= PIPE:
                finalize(done)
                done += 1
            if i >= 8 and done <= i - 1:
                # drain the pipeline early so the tail is short
                finalize(done)
                done += 1
        while done < len(groups):
            finalize(done)
            done += 1

    nc.compile()
    return nc


def _tile_weight(W):
    """[D, H] fp32 -> [HT*P, KT*P] bf16 with [ht*P+p, t*P+h] = W[t*P+p, ht*P+h]."""
    Wb = np.asarray(W, np.float32).astype(NP_BF16)
    return np.ascontiguousarray(
        Wb.reshape(KT, P, HT, P).transpose(2, 1, 0, 3).reshape(HT * P, KT * P))


def kernel(x, h, Wir, bir, Wiz, biz, Win, bin_, Whr, Whz, Whn, bhn,
           ln_scale, ln_bias, Wout, bout):
    global _COMPILED, LAST_RES
    if _COMPILED is None:
        _COMPILED = _build()
    nc = _COMPILED

    x = np.asarray(x, np.float32)
    h = np.asarray(h, np.float32)
    xT = np.ascontiguousarray(x.T).astype(NP_BF16)
    hT = np.ascontiguousarray(h.T).astype(NP_BF16)
    Wout = np.asarray(Wout, np.float32)
    ln_scale = np.asarray(ln_scale, np.float32)
    ln_bias = np.asarray(ln_bias, np.float32)
    woutF = ln_scale[:, None] * Wout
    woutFt = np.ascontiguousarray(
        woutF.astype(NP_BF16).reshape(KT, P, O).transpose(1, 0, 2).reshape(
            P, KT * O))
    boutF = np.asarray(bout, np.float32) + ln_bias @ Wout
    colsum = (ln_scale @ Wout).reshape(1, O)

    def vec_pre(v):
        return np.ascontiguousarray(
            np.asarray(v, np.float32).reshape(-1, P).T)

    common = {
        "Wir": _tile_weight(Wir), "Wiz": _tile_weight(Wiz),
        "Win": _tile_weight(Win), "Whr": _tile_weight(Whr),
        "Whz": _tile_weight(Whz), "Whn": _tile_weight(Whn),
        "woutF": woutFt,
        "bir": vec_pre(bir), "biz": vec_pre(biz),
        "bin": vec_pre(bin_), "bhn": vec_pre(bhn),
        "boutF": vec_pre(boutF), "colsum": colsum.astype(np.float32),
        "ones_col": np.ones((P, 1), np.float32),
        "ones_row": np.ones((1, P), np.float32),
    }
    in_maps = []
    for c in range(NCORES):
        bsl = slice(c * BL, (c + 1) * BL)
        in_maps.append({
            **common,
            "xT": np.ascontiguousarray(xT[:, bsl]),
            "hT": np.ascontiguousarray(hT[:, bsl]),
        })

    res = bass_utils.run_bass_kernel_spmd(nc, in_maps,
                                          core_ids=list(range(NCORES)),
                                          trace=TRACE)
    LAST_RES = res
    new_hT = np.concatenate([np.asarray(res.results[c]["nhT"])
                             for c in range(NCORES)], axis=1)
    outT = np.concatenate([np.asarray(res.results[c]["outT"])
                           for c in range(NCORES)], axis=1)
    new_h = np.ascontiguousarray(new_hT.T).astype(np.float32)
    out = np.ascontiguousarray(outT.T).astype(np.float32)
    return new_h, out


# revision 26
# speedup vs baseline: 1.0322x; 1.0322x over previous
"""GRUCell + LayerNorm readout fused Bass kernel for Trainium2 (8 NeuronCores).

Problem: B=8192, D=H=O=1024 fp32.
    r = sigmoid(x@Wir + bir + h@Whr)
    z = sigmoid(x@Wiz + biz + h@Whz)
    n = tanh(x@Win + bin_ + r*(h@Whn + bhn))
    new_h = (1-z)*n + z*h
    out = (LayerNorm(new_h)*ln_scale + ln_bias) @ Wout + bout

Strategy:
  - Data-parallel over batch: core c gets rows [c*1024, (c+1)*1024); weights
    replicated. No collectives.
  - Everything computed in the transposed domain: activations live as
    [feature, batch] so matmuls take the weights in natural [k, h] layout as
    the stationary operand and xT/hT as the moving operand, and the per-h gate
    biases become per-partition activation biases. Host passes xT/hT and
    transposes the outputs back.
  - All GEMMs in bf16 (1 col/cycle on the PE like f32r, but half the HBM
    traffic and FWL-fast weight loads). Weights are pre-tiled on the host so
    every weight DMA is a contiguous [128, 1024]-per-partition block.
  - Matmuls are emitted k-major, gate-major with both batch chunks adjacent so
    consecutive matmuls share a stationary (one LDWEIGHTS per two matmuls);
    the 8 gate accumulators occupy all 8 PSUM banks.
  - LayerNorm reduces over h (= partition dim): per-tile partials accumulate
    elementwise on DVE, one ones-column reduce matmul per stat at the end.
    The normalize-then-matmul is algebraically folded:
        LN(new_h) @ (ln_scale*Wout) + (ln_bias@Wout + bout)
      = rstd[b]*( new_h@WoutF - mu[b]*colsum[o] ) + boutF[o]
    with WoutF = ln_scale[:,None]*Wout (host), colsum = ln_scale@Wout (host),
    boutF = bout + ln_bias@Wout (host); the mu*colsum term is a K=1 rank-1
    matmul into the same PSUM accumulator.
  - new_h / out are stored to HBM in bf16 and upcast on the host (outputs are
    checked at 2e-2; bf16 rounding is ~4e-3 scale-relative).
"""

import sys
from contextlib import ExitStack

sys.path.insert(0, "/opt/trn_rl_repo")

import ml_dtypes
import numpy as np

import concourse.bacc as bacc
import concourse.mybir as mybir
import concourse.tile as tile
from concourse import bass_isa, bass_utils

B, D, H, O = 8192, 1024, 1024, 1024
NCORES = 8
BL = B // NCORES          # batch rows per core
P = 128                   # partitions
KT = D // P               # contraction tiles (8)
HT = H // P               # h output-partition tiles (8)
OT = O // P               # o output-partition tiles (8)
NB = 2                    # batch chunks per core (free dim 512)
NF = BL // NB             # free dim per chunk (512)
LN_EPS = 1e-6

F32 = mybir.dt.float32
F32R = mybir.dt.float32r
BF16 = mybir.dt.bfloat16
NP_BF16 = ml_dtypes.bfloat16

_COMPILED = None  # compiled Bacc module cache across calls
TRACE = False     # set by test harness to capture an NTFF profile
LAST_RES = None   # BassKernelResults of the last run (for the test harness)

XGATES = ("ir", "iz", "in")
HGATES = ("hr", "hz", "hn")


def _build():
    nc = bacc.Bacc("TRN2", target_bir_lowering=False, debug=False,
                   num_devices=NCORES)

    def din(name, shape, dt=BF16):
        return nc.dram_tensor(name, shape, dt, kind="ExternalInput").ap()

    def dout(name, shape, dt=BF16):
        return nc.dram_tensor(name, shape, dt, kind="ExternalOutput").ap()

    xT_d = din("xT", [D, BL])
    hT_d = din("hT", [H, BL])
    # weights pre-tiled on host: [HT*P, KT*P] with [ht*P+p, t*P+h] =
    # W[t*P+p, ht*P+h]; a per-ht load is contiguous per partition.
    w_d = {g: din(f"W{g}", [HT * P, KT * P]) for g in XGATES + HGATES}
    # [P, KT*O] with [p, t*O+o] = WoutF[t*P+p, o]
    woutF_d = din("woutF", [P, KT * O])
    # bias vectors pre-arranged host-side as [P, HT] (contiguous per partition)
    bir_d = din("bir", [P, HT], F32)
    biz_d = din("biz", [P, HT], F32)
    bin_d = din("bin", [P, HT], F32)
    bhn_d = din("bhn", [P, HT], F32)
    boutF_d = din("boutF", [P, OT], F32)
    colsum_d = din("colsum", [1, O], F32R)
    ones_col_d = din("ones_col", [P, 1], F32R)
    ones_row_d = din("ones_row", [1, P], F32R)

    nhT_d = dout("nhT", [H, BL])
    outT_d = dout("outT", [O, BL])

    with tile.TileContext(nc) as tc, ExitStack() as ctx:
        singles = ctx.enter_context(tc.tile_pool(name="singles", bufs=1))
        wpool = ctx.enter_context(tc.tile_pool(name="wpool", bufs=2))
        gates = ctx.enter_context(tc.tile_pool(name="gates", bufs=1))
        rows = ctx.enter_context(tc.tile_pool(name="rows", bufs=1))
        ps = ctx.enter_context(tc.tile_pool(name="ps", bufs=1, space="PSUM"))

        # ---- resident inputs, DMA-ordered to feed the PE ramp ---------------
        def kslice_tile(prefix, k):
            return singles.tile([P, BL], BF16, tag=f"{prefix}{k}",
                                name=f"{prefix}{k}")

        def load_w(g, ht):
            t = wpool.tile([P, KT, P], BF16, tag=f"w{g}", name=f"w{g}_{ht}")
            nc.sync.dma_start(t[:], w_d[g][ht * P:(ht + 1) * P, :].rearrange(
                "p (t h) -> p t h", t=KT))
            return t

        xT_sb, hT_sb = [], []
        for k in range(KT):
            xT_sb.append(kslice_tile("xk", k))
            hT_sb.append(kslice_tile("hk", k))

        # input slices ride the Scalar engine's DMA queue and weights the
        # Sync queue: two issue streams (~0.6us per DMA instruction) so the
        # prologue isn't serialized on a single queue.
        nc.scalar.dma_start(xT_sb[0][:], xT_d[0:P, :])
        w0 = {g: load_w(g, 0) for g in XGATES}
        for k in range(1, KT):
            nc.scalar.dma_start(xT_sb[k][:], xT_d[k * P:(k + 1) * P, :])
        w0["hr"] = load_w("hr", 0)
        w0["hz"] = load_w("hz", 0)
        w0["hn"] = load_w("hn", 0)
        for k in range(KT):
            nc.scalar.dma_start(hT_sb[k][:], hT_d[k * P:(k + 1) * P, :])

        def load_vec(ap_d, n, tag):
            t = singles.tile([P, n // P], F32, tag=tag, name=tag)
            nc.scalar.dma_start(t[:], ap_d)
            return t

        bir_sb = load_vec(bir_d, H, "bir_sb")
        biz_sb = load_vec(biz_d, H, "biz_sb")
        bin_sb = load_vec(bin_d, H, "bin_sb")
        bhn_sb = load_vec(bhn_d, H, "bhn_sb")
        boutF_sb = load_vec(boutF_d, O, "boutF_sb")
        colsum_sb = singles.tile([1, O], F32R)
        nc.scalar.dma_start(colsum_sb[:], colsum_d)
        ones_col = singles.tile([P, 1], F32R)
        nc.scalar.dma_start(ones_col[:], ones_col_d)
        ones_row = singles.tile([1, P], F32R)
        nc.scalar.dma_start(ones_row[:], ones_row_d)
        eps_sb = singles.tile([1, 1], F32)
        nc.vector.memset(eps_sb[:], LN_EPS)

        # ---- PE warmup: junk matmuls on a zeroed tile keep the HAM activity
        # monitor busy while the first weights/inputs stream in, so the real
        # matmul stream starts at the full 2.4GHz clock.
        warm_sb = singles.tile([P, NF], BF16, tag="warm", name="warm_sb")
        nc.vector.memset(warm_sb[:], 0.0)
        warm_ps = ps.tile([P, NF], F32, tag="gh1", name="warm_ps")
        for _ in range(16):
            nc.tensor.matmul(warm_ps[:], warm_sb[:, 0:P], warm_sb[:],
                             start=True, stop=True, skip_group_check=True)

        # per-(ht, bc) new_h tiles: tile-granular deps keep the readout
        # matmuls from waiting on the other batch-chunk's epilogue
        new_hT_sb = [[singles.tile([P, NF], BF16, tag=f"nh{ht}_{bc}",
                                   name=f"nh{ht}_{bc}") for bc in range(NB)]
                     for ht in range(HT)]
        s_acc = [singles.tile([P, NF], F32R, tag=f"s_acc{bc}",
                              name=f"s_acc{bc}") for bc in range(NB)]
        q_acc = [singles.tile([P, NF], F32R, tag=f"q_acc{bc}",
                              name=f"q_acc{bc}") for bc in range(NB)]

        # ---- phase 1: gates + new_h -----------------------------------------
        woutF_sb = singles.tile([P, KT, O], BF16)

        for ht in range(HT):
            hs = slice(ht * P, (ht + 1) * P)
            w_sb = w0 if ht == 0 else {g: load_w(g, ht)
                                       for g in XGATES + HGATES}
            if ht == 2:
                # readout weights: resident; loaded after the ramp-critical
                # input/gate-weight prefetches are in flight
                nc.sync.dma_start(woutF_sb[:], woutF_d.rearrange(
                    "p (t o) -> p t o", t=KT))

            pr = [ps.tile([P, NF], F32, tag=f"r{bc}", name=f"pr{bc}_{ht}")
                  for bc in range(NB)]
            pz = [ps.tile([P, NF], F32, tag=f"z{bc}", name=f"pz{bc}_{ht}")
                  for bc in range(NB)]
            pgi = [ps.tile([P, NF], F32, tag=f"gi{bc}", name=f"pgi{bc}_{ht}")
                   for bc in range(NB)]
            pgh = [ps.tile([P, NF], F32, tag=f"gh{bc}", name=f"pgh{bc}_{ht}")
                   for bc in range(NB)]

            bsl = [slice(bc * NF, (bc + 1) * NF) for bc in range(NB)]

            # bc-major: all of bc0's matmuls, then bc1's. bc0's PSUM groups
            # stop ~5us before the ht ends, so its activation/epilogue chain
            # (and at ht=7 the nh tile the readout needs) hides under bc1's
            # matmuls instead of stalling the next block. Exception: ht=0
            # runs k-interleaved — it consumes input slices at half the rate,
            # which is what the still-ramping DMA stream can sustain.
            if ht == 0:
                for k in range(KT):
                    for g, acc in (("ir", pr), ("iz", pz), ("in", pgi)):
                        for bc in range(NB):
                            nc.tensor.matmul(
                                acc[bc][:], w_sb[g][:, k, :],
                                xT_sb[k][:, bsl[bc]], start=(k == 0),
                                stop=(k == KT - 1 and g == "in"))
                for k in range(KT):
                    for g, acc in (("hr", pr), ("hz", pz), ("hn", pgh)):
                        for bc in range(NB):
                            nc.tensor.matmul(
                                acc[bc][:], w_sb[g][:, k, :],
                                hT_sb[k][:, bsl[bc]],
                                start=(k == 0 and g == "hn"),
                                stop=(k == KT - 1))
            else:
                for bc in range(NB):
                    for k in range(KT):
                        nc.tensor.matmul(pr[bc][:], w_sb["ir"][:, k, :],
                                         xT_sb[k][:, bsl[bc]],
                                         start=(k == 0), stop=False)
                        nc.tensor.matmul(pz[bc][:], w_sb["iz"][:, k, :],
                                         xT_sb[k][:, bsl[bc]],
                                         start=(k == 0), stop=False)
                        nc.tensor.matmul(pgi[bc][:], w_sb["in"][:, k, :],
                                         xT_sb[k][:, bsl[bc]],
                                         start=(k == 0), stop=(k == KT - 1))
                for bc in range(NB):
                    for k in range(KT):
                        nc.tensor.matmul(pr[bc][:], w_sb["hr"][:, k, :],
                                         hT_sb[k][:, bsl[bc]],
                                         start=False, stop=(k == KT - 1))
                        nc.tensor.matmul(pz[bc][:], w_sb["hz"][:, k, :],
                                         hT_sb[k][:, bsl[bc]],
                                         start=False, stop=(k == KT - 1))
                        nc.tensor.matmul(pgh[bc][:], w_sb["hn"][:, k, :],
                                         hT_sb[k][:, bsl[bc]],
                                         start=(k == 0), stop=(k == KT - 1))

            for bc in range(NB):
                bs = bsl[bc]
                r_sb = gates.tile([P, NF], F32, tag="r_act")
                nc.scalar.activation(r_sb[:], pr[bc][:],
                                     mybir.ActivationFunctionType.Sigmoid,
                                     bias=bir_sb[:, ht:ht + 1])
                z_sb = gates.tile([P, NF], F32, tag="z_act")
                nc.scalar.activation(z_sb[:], pz[bc][:],
                                     mybir.ActivationFunctionType.Sigmoid,
                                     bias=biz_sb[:, ht:ht + 1])

                t_sb = gates.tile([P, NF], F32, tag="t")
                nc.vector.tensor_scalar(t_sb[:], pgh[bc][:],
                                        bhn_sb[:, ht:ht + 1],
                                        None, mybir.AluOpType.add)
                nc.vector.tensor_mul(t_sb[:], t_sb[:], r_sb[:])
                nc.vector.tensor_add(t_sb[:], t_sb[:], pgi[bc][:])
                n_sb = gates.tile([P, NF], F32, tag="r_act", name="n_sb")
                nc.scalar.activation(n_sb[:], t_sb[:],
                                     mybir.ActivationFunctionType.Tanh,
                                     bias=bin_sb[:, ht:ht + 1])

                u_sb = gates.tile([P, NF], F32, tag="u")
                nc.vector.tensor_tensor(u_sb[:], hT_sb[ht][:, bs],
                                        n_sb[:], mybir.AluOpType.subtract)
                nc.vector.tensor_mul(u_sb[:], z_sb[:], u_sb[:])
                nh = new_hT_sb[ht][bc][:]
                nc.vector.tensor_add(nh, n_sb[:], u_sb[:])

                # LN stat partials: elementwise accumulate over h-tiles (DVE),
                # cross-partition reduce later via a ones-column matmul.
                sq_sb = gates.tile([P, NF], F32R, tag="t", name="sq_sb")
                if ht == 0:
                    nc.vector.tensor_copy(s_acc[bc][:], nh)
                    nc.scalar.activation(q_acc[bc][:], nh,
                                         mybir.ActivationFunctionType.Square)
                else:
                    nc.vector.tensor_tensor(s_acc[bc][:],
                                            s_acc[bc][:].bitcast(F32), nh,
                                            mybir.AluOpType.add)
                    nc.scalar.activation(sq_sb[:], nh,
                                         mybir.ActivationFunctionType.Square)
                    nc.vector.tensor_tensor(q_acc[bc][:],
                                            q_acc[bc][:].bitcast(F32),
                                            sq_sb[:].bitcast(F32),
                                            mybir.AluOpType.add)

                # stores go through GpSimd's DMA queue so they never
                # head-of-line-block weight loads on the Sync queue
                nc.gpsimd.dma_start(nhT_d[hs, bs], nh)

        # ---- phase 2: LN scale factors + readout ----------------------------
        # bc=0 groups run first so the bc=1 stats chain hides under them; the
        # stats reduce-matmuls, broadcast matmuls, rank-1s, and epilogues are
        # all software-pipelined into the main matmul stream so the in-order
        # PE never stalls on the stats chain or cools down (HAM).
        red_tags = ("gi0", "gi1", "gi0", "gi1")
        nmu_row = {}
        rstd_row = {}
        rstd_bc = {}

        def emit_stats(bc):
            psum_s = ps.tile([1, NF], F32, tag=red_tags[2 * bc],
                             name=f"psum_s{bc}")
            nc.tensor.matmul(psum_s[:], ones_col[:], s_acc[bc][:],
                             start=True, stop=True)
            psum_q = ps.tile([1, NF], F32, tag=red_tags[2 * bc + 1],
                             name=f"psum_q{bc}")
            nc.tensor.matmul(psum_q[:], ones_col[:], q_acc[bc][:],
                             start=True, stop=True)

            nmu = rows.tile([1, NF], F32R, tag=f"nmu{bc}", name=f"nmu{bc}")
            nc.vector.tensor_scalar_mul(nmu[:], psum_s[:], -1.0 / H)
            nmu_row[bc] = nmu

            mu2 = gates.tile([1, NF], F32, tag="t", name=f"mu2_{bc}")
            nc.vector.tensor_mul(mu2[:], nmu[:].bitcast(F32), nmu[:].bitcast(F32))
            var = gates.tile([1, NF], F32, tag="u", name=f"var_{bc}")
            nc.vector.scalar_tensor_tensor(var[:], psum_q[:], 1.0 / H, mu2[:],
                                           mybir.AluOpType.mult,
                                           mybir.AluOpType.subtract)
            nc.scalar.activation(var[:], var[:],
                                 mybir.ActivationFunctionType.Sqrt,
                                 bias=eps_sb[:])
            rrow = rows.tile([1, NF], F32, tag=f"rstdr{bc}",
                             name=f"rstd{bc}")
            nc.vector.reciprocal_approx_fast(rrow[:], var[:])
            rstd_row[bc] = rrow

        po_tags = ("r0", "z0", "r1", "z1", "gh0", "gh1")
        PIPE = 5
        groups = [(ot, bc) for bc in range(NB) for ot in range(OT)]
        pending = {}

        def finalize(i):
            ot, bc = groups[i]
            po = pending.pop(i)
            os_ = slice(ot * P, (ot + 1) * P)
            bs = slice(bc * NF, (bc + 1) * NF)
            # -= mu[b] * colsum[o]  (rank-1, K=1)
            nc.tensor.matmul(po[:], colsum_sb[0:1, os_], nmu_row[bc][:],
                             start=False, stop=True)
            o_sb = gates.tile([P, NF], BF16, tag=("ob0", "ob1", "ob2")[i % 3],
                              name=f"o_{ot}_{bc}")
            nc.vector.tensor_mul(o_sb[:], po[:], rstd_bc[bc][:])
            nc.vector.tensor_scalar(o_sb[:], o_sb[:],
                                    boutF_sb[:, ot:ot + 1], None,
                                    mybir.AluOpType.add)
            # sync queue is idle during phase 2; keeps the gpsimd drain short
            nc.sync.dma_start(outT_d[os_, bs], o_sb[:])

        def emit_pb(bc):
            # gpsimd SBUF->SBUF broadcast: no PE or Vector time
            rb = rows.tile([P, NF], F32, tag=f"rstd_bc{bc}",
                           name=f"rstd_bc{bc}")
            nc.gpsimd.partition_broadcast(rb[:], rstd_row[bc][:])
            rstd_bc[bc] = rb

        done = 0
        for i, (ot, bc) in enumerate(groups):
            po = ps.tile([P, NF], F32, tag=po_tags[i % len(po_tags)],
                         name=f"po_{ot}_{bc}")
            for k in range(HT):
                nc.tensor.matmul(po[:], woutF_sb[:, k, ot * P:(ot + 1) * P],
                                 new_hT_sb[k][bc][:],
                                 start=(k == 0), stop=False)
            pending[i] = po
            if i == 1:
                emit_stats(0)
            elif i == 3:
                emit_stats(1)
            elif i == 5:
                emit_pb(0)
            elif i == 7:
                emit_pb(1)
            if i >= PIPE:
                finalize(done)
                done += 1
            if i >= 8 and done <= i - 1:
                # drain the pipeline early so the tail is short
                finalize(done)
                done += 1
        while done < len(groups):
            finalize(done)
            done += 1

    nc.compile()
    return nc


def _tile_weight(W):
    """[D, H] fp32 -> [HT*P, KT*P] bf16 with [ht*P+p, t*P+h] = W[t*P+p, ht*P+h]."""
    Wb = np.asarray(W, np.float32).astype(NP_BF16)
    return np.ascontiguousarray(
        Wb.reshape(KT, P, HT, P).transpose(2, 1, 0, 3).reshape(HT * P, KT * P))


def kernel(x, h, Wir, bir, Wiz, biz, Win, bin_, Whr, Whz, Whn, bhn,
           ln_scale, ln_bias, Wout, bout):
    global _COMPILED, LAST_RES
    if _COMPILED is None:
        _COMPILED = _build()
    nc = _COMPILED

    x = np.asarray(x, np.float32)
    h = np.asarray(h, np.float32)
    xT = np.ascontiguousarray(x.T).astype(NP_BF16)
    hT = np.ascontiguousarray(h.T).astype(NP_BF16)
    Wout = np.asarray(Wout, np.float32)
    ln_scale = np.asarray(ln_scale, np.float32)
    ln_bias = np.asarray(ln_bias, np.float32)
    woutF = ln_scale[:, None] * Wout
    woutFt = np.ascontiguousarray(
        woutF.astype(NP_BF16).reshape(KT, P, O).transpose(1, 0, 2).reshape(
            P, KT * O))
    boutF = np.asarray(bout, np.float32) + ln_bias @ Wout
    colsum = (ln_scale @ Wout).reshape(1, O)

    def vec_pre(v):
        return np.ascontiguousarray(
            np.asarray(v, np.float32).reshape(-1, P).T)

    common = {
        "Wir": _tile_weight(Wir), "Wiz": _tile_weight(Wiz),
        "Win": _tile_weight(Win), "Whr": _tile_weight(Whr),
        "Whz": _tile_weight(Whz), "Whn": _tile_weight(Whn),
        "woutF": woutFt,
        "bir": vec_pre(bir), "biz": vec_pre(biz),
        "bin": vec_pre(bin_), "bhn": vec_pre(bhn),
        "boutF": vec_pre(boutF), "colsum": colsum.astype(np.float32),
        "ones_col": np.ones((P, 1), np.float32),
        "ones_row": np.ones((1, P), np.float32),
    }
    in_maps = []
    for c in range(NCORES):
        bsl = slice(c * BL, (c + 1) * BL)
        in_maps.append({
            **common,
            "xT": np.ascontiguousarray(xT[:, bsl]),
            "hT": np.ascontiguousarray(hT[:, bsl]),
        })

    res = bass_utils.run_bass_kernel_spmd(nc, in_maps,
                                          core_ids=list(range(NCORES)),
                                          trace=TRACE)
    LAST_RES = res
    new_hT = np.concatenate([np.asarray(res.results[c]["nhT"])
                             for c in range(NCORES)], axis=1)
    outT = np.concatenate([np.asarray(res.results[c]["outT"])
                           for c in range(NCORES)], axis=1)
    new_h = np.ascontiguousarray(new_hT.T).astype(np.float32)
    out = np.ascontiguousarray(outT.T).astype(np.float32)
    return new_h, out
